# revision 8
# baseline (speedup 1.0000x reference)
"""Causal self-attention (B=2, T=4096, C=768, H=12) on 8 trn2 NeuronCores.

Sharding: core c handles batch b = c//4 and the 3 heads of head-group
hg = c%4 (tensor parallel over heads, data parallel over batch).  Each core
computes the qkv projection for its heads, causal attention, and a partial
output projection; the host sums the 4 per-head-group partials per batch.

Device notes (v2):
  - Scores are computed transposed (S^T[tk, tq] = K Q^T).  h0/h1 run as a
    row-tiled pair (array rows 0-63 / 64-127) writing the two 512-halves of
    one 2-bank PSUM tile, so the pair issues adjacently and overlaps on the
    PE.  h2 self-pairs via a duplicated q/k slab.
  - exp is split across three engines: ACT uses the exact table exp; DVE
    and GpSimd use a Schraudolph-style approximation (u16 = round(s*A + B)
    bitcast as bf16), whose systematic part cancels in the softmax
    normalization.  Masks fold in afterwards via one strided multiply.
  - PV runs col-tiled: h0 -> PSUM partitions 0-63 (tile col 0), h1 ->
    64-127 (col 64) concurrently.  Softmax denominators are M=1 col-tiled
    matmuls against a ones column riding the spare 32-wide col strips.
  - Diagonal h0/h1 blocks only compute/exp the valid q-range (j >= 128*poff).
"""

import ml_dtypes
import numpy as np

import concourse.bass as bass
import concourse.mybir as mybir
import concourse.tile as tile
from concourse import bacc

B, T, C, H, HD = 2, 4096, 768, 12, 64
F32 = mybir.dt.float32
BF16 = mybir.dt.bfloat16
U16 = mybir.dt.uint16
N_CORES = 8
AF = mybir.ActivationFunctionType
OP = mybir.AluOpType

LOG2E = 1.4426950408889634
SCH_A = 0.125 * 128.0 * LOG2E      # scale folded with the 1/sqrt(64) factor
SCH_B = 127.0 * 128.0 - 7.4        # schraudolph bias (bf16 exponent offset)

# per-slot exp engine pattern: A=ACT exact, D=DVE schraud (GpSimd cannot
# read PSUM, so it gets the SBUF-only work: masks + normalization muls)
EXP_PATTERN = "AD"


def build_nc(seq_len: int = T) -> bass.Bass:
    assert seq_len % 512 == 0
    TCH = seq_len // 512   # 512-wide t-chunks
    TB = seq_len // 128    # 128-wide t-blocks

    nc = bacc.Bacc(num_devices=N_CORES)

    xT = nc.dram_tensor("xT", (C, seq_len), BF16, kind="ExternalInput").ap()
    wqkT = nc.dram_tensor("wqkT", (C, 384), BF16, kind="ExternalInput").ap()
    wvT = nc.dram_tensor("wvT", (C, 192), BF16, kind="ExternalInput").ap()
    wpT = nc.dram_tensor("wpT", (192, C), BF16, kind="ExternalInput").ap()
    out = nc.dram_tensor("out", (seq_len, C), F32, kind="ExternalOutput").ap()

    exp_slot = [0]

    with tile.TileContext(nc) as tc:
        with (
            tc.tile_pool(name="const", bufs=1) as const,
            tc.tile_pool(name="persist", bufs=1) as persist,
            tc.tile_pool(name="xt", bufs=2) as xtpool,
            tc.tile_pool(name="p01", bufs=8) as p01pool,
            tc.tile_pool(name="p2", bufs=18) as p2pool,
            tc.tile_pool(name="small", bufs=2) as spool,
            tc.tile_pool(name="ev", bufs=2) as evpool,
            tc.tile_pool(name="osb", bufs=2) as osbpool,
            tc.tile_pool(name="ps", bufs=2, space="PSUM") as pspool,
            tc.tile_pool(name="pso", bufs=1, space="PSUM") as psopool,
            tc.tile_pool(name="pso2", bufs=1, space="PSUM") as pso2pool,
            tc.tile_pool(name="pden", bufs=1, space="PSUM") as pdenpool,
            tc.tile_pool(name="pp", bufs=1, space="PSUM") as pppool,
        ):
            # ---- constants / weights ----
            wqk_sb = const.tile([128, 6, 384], BF16, tag="wqk")
            nc.sync.dma_start(wqk_sb, wqkT.rearrange("(cc p) o -> p cc o", p=128))
            wv_sb = const.tile([128, 6, 192], BF16, tag="wv")
            nc.sync.dma_start(wv_sb, wvT.rearrange("(cc p) o -> p cc o", p=128))
            wp0_sb = const.tile([128, 768], BF16, tag="wp0")
            nc.sync.dma_start(wp0_sb, wpT[0:128, :])
            wp1_sb = const.tile([128, 768], BF16, tag="wp1")
            nc.vector.memset(wp1_sb[64:128, :], 0.0)
            nc.sync.dma_start(wp1_sb[0:64, :], wpT[128:192, :])
            ones_sb = const.tile([128, 1], BF16, tag="ones")
            nc.gpsimd.memset(ones_sb, 1.0)

            # emask2[i, t, col] = 1.0 if col >= i + 384 else 0.0 (two copies
            # so one 3D-AP multiply masks both 512-halves of a pt tile).
            emask2 = const.tile([128, 2, 896], BF16, tag="emask2")
            nc.gpsimd.memset(emask2, 1.0)
            for hm in range(2):
                nc.gpsimd.affine_select(
                    out=emask2[:, hm, :], in_=emask2[:, hm, :],
                    compare_op=mybir.AluOpType.is_ge,
                    fill=0.0, base=-384, pattern=[[1, 896]], channel_multiplier=-1,
                )

            # ---- persistent activations ----
            # qT/kT slab0: h0 @ partitions 0-63, h1 @ 64-127.
            # slab1: h2 duplicated to both halves (self-pairing).
            qT_sb = persist.tile([128, 2, seq_len], BF16, tag="qT")
            kT_sb = persist.tile([128, 2, seq_len], BF16, tag="kT")
            # v for all 3 heads: [t-partition, kb, 3*64]
            v_sb = persist.tile([128, TB, 192], BF16, tag="v")
            # attention output, transposed: slab0 = [h0 | h1], slab1 = [h2 | 0]
            outT_sb = persist.tile([128, 2, seq_len], BF16, tag="outT")
            nc.vector.memset(outT_sb[64:128, 1, :], 0.0)

            # ---- phase 1: qkv projection ----
            for tci in range(TCH):
                tcs = slice(tci * 512, (tci + 1) * 512)
                xt = xtpool.tile([128, 6, 512], BF16, tag="xt")
                for cc in range(6):
                    nc.sync.dma_start(
                        xt[:, cc, :], xT[cc * 128:(cc + 1) * 128, tcs]
                    )
                # q/k channels: m0=[q_h0|q_h1], m1=[k_h0|k_h1], m2=[q_h2|k_h2]
                for m in range(3):
                    ps = pspool.tile([128, 2, 512], F32, tag="ps", name="psqk")
                    for cc in range(6):
                        nc.tensor.matmul(
                            ps[:, 0, :],
                            lhsT=wqk_sb[:, cc, m * 128:(m + 1) * 128],
                            rhs=xt[:, cc, :],
                            start=(cc == 0), stop=(cc == 5),
                        )
                    if m == 0:
                        nc.scalar.copy(qT_sb[:, 0, tcs], ps[:, 0, :])
                    elif m == 1:
                        nc.scalar.copy(kT_sb[:, 0, tcs], ps[:, 0, :])
                    else:
                        # h2: land q at 0-63 / k at 64-127, then duplicate to
                        # the opposite half via SBUF->SBUF DMA.
                        nc.scalar.copy(qT_sb[0:64, 1, tcs], ps[0:64, 0, :])
                        nc.scalar.copy(kT_sb[64:128, 1, tcs], ps[64:128, 0, :])
                        nc.sync.dma_start(qT_sb[64:128, 1, tcs], qT_sb[0:64, 1, tcs])
                        nc.sync.dma_start(kT_sb[0:64, 1, tcs], kT_sb[64:128, 1, tcs])
                # v channels
                for tb in range(4):
                    psv = pspool.tile([128, 2, 512], F32, tag="ps", name="psv")
                    for cc in range(6):
                        nc.tensor.matmul(
                            psv[:, 0, 0:192],
                            lhsT=xt[:, cc, tb * 128:(tb + 1) * 128],
                            rhs=wv_sb[:, cc, :],
                            start=(cc == 0), stop=(cc == 5),
                        )
                    nc.vector.tensor_copy(
                        v_sb[:, tci * 4 + tb, :], psv[:, 0, 0:192]
                    )

            # ---- phase 2: attention (pipelined per k-block) ----
            def emit_exp(sp, pt, j0, diag_poff):
                """exp of sp[:, :, j0:512] -> pt (same region); then mask."""
                eng = EXP_PATTERN[exp_slot[0] % len(EXP_PATTERN)]
                exp_slot[0] += 1
                w = 512 - j0
                if eng == 'A':
                    nc.scalar.activation(
                        pt[:, :, j0:512], sp[:, :, j0:512], AF.Exp, scale=0.125)
                else:
                    nc.vector.tensor_scalar(
                        pt.bitcast(U16)[:, :, j0:512], sp[:, :, j0:512],
                        SCH_A, SCH_B, OP.mult, OP.add,
                    )
                if diag_poff is not None:
                    nc.gpsimd.tensor_mul(
                        pt[:, :, j0:512], pt[:, :, j0:512],
                        emask2[:, :, 384:384 + w],
                    )

            def st_slot_h01(qc, kb, j0):
                """S^T row-tiled pair for h0/h1, one kb -> pt [p_h0 | p_h1]."""
                poff = kb - 4 * qc
                sp = pspool.tile([128, 2, 512], F32, tag="ps", name="sp")
                for t, base in ((0, 0), (1, 64)):
                    nc.tensor.matmul(
                        sp[:, t, j0:512],
                        lhsT=kT_sb[base:base + 64, 0, kb * 128:(kb + 1) * 128],
                        rhs=qT_sb[base:base + 64, 0, qc * 512 + j0:(qc + 1) * 512],
                        start=True, stop=True,
                    )
                pt = p01pool.tile([128, 2, 512], BF16, tag="p01", name="pt")
                emit_exp(sp, pt, j0, poff if poff >= 0 else None)
                return pt

            def st_slot_h2(qc, j, half, qcs):
                """S^T pair for h2 with itself: kb j (rows 0-63, slab 1) and
                kb half+j (rows 64-127, slab 1) -> pt [p_j | p_half+j]."""
                kb2 = half + j
                sp = pspool.tile([128, 2, 512], F32, tag="ps", name="sp")
                nc.tensor.matmul(
                    sp[:, 0, :],
                    lhsT=kT_sb[0:64, 1, j * 128:(j + 1) * 128],
                    rhs=qT_sb[0:64, 1, qcs],
                    start=True, stop=True,
                )
                nc.tensor.matmul(
                    sp[:, 1, :],
                    lhsT=kT_sb[64:128, 1, kb2 * 128:(kb2 + 1) * 128],
                    rhs=qT_sb[64:128, 1, qcs],
                    start=True, stop=True,
                )
                pt = p2pool.tile([128, 2, 512], BF16, tag="p2", name="pt2")
                emit_exp(sp, pt, 0, None)
                for t, kbx in ((0, j), (1, kb2)):
                    poff = kbx - 4 * qc
                    if poff >= 0:
                        off = 384 - 128 * poff
                        nc.gpsimd.tensor_mul(
                            pt[:, t, :], pt[:, t, :], emask2[:, 0, off:off + 512])
                return pt

            def proj_chunk(qc):
                for tb in range(4 * qc, 4 * qc + 4):
                    tbs = slice(tb * 128, (tb + 1) * 128)
                    ob = osbpool.tile([128, 768], F32, tag="osb")
                    for n0, nsz in ((0, 512), (512, 256)):
                        pp = pppool.tile([128, 512], F32, tag="pp", name="pp")
                        nc.tensor.matmul(
                            pp[:, :nsz],
                            lhsT=outT_sb[:, 0, tbs],
                            rhs=wp0_sb[:, n0:n0 + nsz],
                            start=True, stop=False,
                        )
                        nc.tensor.matmul(
                            pp[:, :nsz],
                            lhsT=outT_sb[:, 1, tbs],
                            rhs=wp1_sb[:, n0:n0 + nsz],
                            start=False, stop=True,
                        )
                        nc.vector.tensor_copy(ob[:, n0:n0 + nsz], pp[:, :nsz])
                    nc.sync.dma_start(out[tbs, :], ob)

            for qc in range(TCH):
                qcs = slice(qc * 512, (qc + 1) * 512)
                nkb = 4 * (qc + 1)
                half = nkb // 2

                pso = psopool.tile([128, 512], F32, tag="pso", name="pso")
                pso2 = pso2pool.tile([128, 512], F32, tag="pso2", name="pso2")
                pden = pdenpool.tile([128, 512], F32, tag="pden", name="pden")

                pt2_tiles = {}
                for i in range(nkb):
                    poff = i - 4 * qc
                    j0 = 128 * poff if poff > 0 else 0
                    last = (i == nkb - 1)
                    # stage A
                    pt01 = st_slot_h01(qc, i, j0)
                    if i < half:
                        pt2_tiles[i] = st_slot_h2(qc, i, half, qcs)
                        pt2, t2 = pt2_tiles[i], 0
                    else:
                        pt2, t2 = pt2_tiles[i - half], 1
                    # B1: PV h0 || h1 (col strips 0 / 64)
                    nc.tensor.matmul(
                        pso[0:64, j0:512], lhsT=v_sb[:, i, 0:64],
                        rhs=pt01[:, 0, j0:512],
                        start=(i == 0), stop=last, tile_position=(0, 0))
                    nc.tensor.matmul(
                        pso[64:128, j0:512], lhsT=v_sb[:, i, 64:128],
                        rhs=pt01[:, 1, j0:512],
                        start=(i == 0), stop=last, tile_position=(0, 64))
                    # B2: PV h2 (cols 0-63) || den_h0 (col 64) || den_h1 (col 96)
                    nc.tensor.matmul(
                        pso2[0:64, :], lhsT=v_sb[:, i, 128:192],
                        rhs=pt2[:, t2, :],
                        start=(i == 0), stop=last, tile_position=(0, 0))
                    nc.tensor.matmul(
                        pso2[64:65, j0:512], lhsT=ones_sb[:, :],
                        rhs=pt01[:, 0, j0:512],
                        start=(i == 0), stop=last, tile_position=(0, 64))
                    nc.tensor.matmul(
                        pso2[96:97, j0:512], lhsT=ones_sb[:, :],
                        rhs=pt01[:, 1, j0:512],
                        start=(i == 0), stop=last, tile_position=(0, 96))
                    # B3: den_h2, 4 partial chains on strips 0/32/64/96 of pden
                    strip = 32 * (i % 4)
                    nc.tensor.matmul(
                        pden[strip:strip + 1, :], lhsT=ones_sb[:, :],
                        rhs=pt2[:, t2, :],
                        start=(i < 4), stop=(i >= nkb - 4),
                        tile_position=(0, strip))

                if qc > 0:
                    proj_chunk(qc - 1)

                # ---- normalization ----
                ev01 = evpool.tile([128, 512], F32, tag="ev01")
                nc.vector.tensor_copy(ev01, pso)
                ev2 = evpool.tile([128, 512], F32, tag="ev2")
                nc.vector.tensor_copy(ev2[0:97, :], pso2[0:97, :])
                evd = evpool.tile([128, 512], F32, tag="evd")
                nc.vector.tensor_copy(evd[0:97, :], pden[0:97, :])
                # split denominator rows across 64 partitions (fast reciprocal)
                lsp = spool.tile([64, 6, 8], F32, tag="lsp")
                nc.sync.dma_start(lsp[:, 0, :], ev2[64:65, :])   # den h0
                nc.sync.dma_start(lsp[:, 1, :], ev2[96:97, :])   # den h1
                for j in range(4):                               # den h2 parts
                    nc.sync.dma_start(lsp[:, 2 + j, :], evd[32 * j:32 * j + 1, :])
                nc.vector.tensor_add(lsp[:, 2, :], lsp[:, 2, :], lsp[:, 3, :])
                nc.vector.tensor_add(lsp[:, 4, :], lsp[:, 4, :], lsp[:, 5, :])
                nc.vector.tensor_add(lsp[:, 2, :], lsp[:, 2, :], lsp[:, 4, :])
                lrec = spool.tile([64, 3, 8], F32, tag="lrec")
                nc.vector.reciprocal(lrec, lsp[:, 0:3, :])
                lrow = spool.tile([1, 3, 512], F32, tag="lrow")
                for j in range(3):
                    nc.sync.dma_start(lrow[:, j, :], lrec[:, j, :])
                # bc128: rows 0-63 = 1/den_h0, rows 64-127 = 1/den_h1
                bc128 = spool.tile([128, 512], F32, tag="bc128")
                nc.gpsimd.partition_broadcast(bc128[0:64, :], lrow[:, 0, :])
                bct = spool.tile([64, 512], F32, tag="bct")
                nc.gpsimd.partition_broadcast(bct, lrow[:, 1, :])
                nc.sync.dma_start(bc128[64:128, :], bct)
                bc2 = spool.tile([64, 512], F32, tag="bc2")
                nc.gpsimd.partition_broadcast(bc2, lrow[:, 2, :])
                nc.gpsimd.tensor_mul(outT_sb[:, 0, qcs], ev01, bc128)
                nc.gpsimd.tensor_mul(outT_sb[0:64, 1, qcs], ev2[0:64, :], bc2)

            proj_chunk(TCH - 1)

    nc.compile()
    return nc


_NC_CACHE: dict[int, bass.Bass] = {}


def get_nc(seq_len: int) -> bass.Bass:
    if seq_len not in _NC_CACHE:
        _NC_CACHE[seq_len] = build_nc(seq_len)
    return _NC_CACHE[seq_len]


def make_in_maps(x: np.ndarray, w_attn: np.ndarray, w_proj: np.ndarray):
    """Per-core input dicts. Core c: batch c//4, head group c%4 (3 heads)."""
    bf16 = ml_dtypes.bfloat16
    in_maps = []
    for c in range(N_CORES):
        b, hg = divmod(c, 4)
        q = w_attn[192 * hg: 192 * hg + 192]
        k = w_attn[768 + 192 * hg: 768 + 192 * hg + 192]
        v = w_attn[1536 + 192 * hg: 1536 + 192 * hg + 192]
        wqk = np.concatenate([q[0:128], k[0:128], q[128:192], k[128:192]], axis=0)
        in_maps.append({
            "xT": np.ascontiguousarray(x[b].T).astype(bf16),
            "wqkT": np.ascontiguousarray(wqk.T).astype(bf16),
            "wvT": np.ascontiguousarray(v.T).astype(bf16),
            "wpT": np.ascontiguousarray(
                w_proj[:, 192 * hg: 192 * hg + 192].T
            ).astype(bf16),
        })
    return in_maps


def run_on_cores(x, w_attn, w_proj, trace: bool = False):
    from concourse.bass_utils import run_bass_kernel_spmd

    x = np.asarray(x, dtype=np.float32)
    w_attn = np.asarray(w_attn, dtype=np.float32)
    w_proj = np.asarray(w_proj, dtype=np.float32)
    nc = get_nc(x.shape[1])
    in_maps = make_in_maps(x, w_attn, w_proj)
    res = run_bass_kernel_spmd(
        nc, in_maps, core_ids=list(range(N_CORES)), trace=trace
    )
    outs = [r["out"] for r in res.results]
    full = np.stack(
        [sum(outs[4 * b + hg] for hg in range(4)) for b in range(B)], axis=0
    )
    return full, res


def kernel(x, w_attn, w_proj):
    full, _ = run_on_cores(x, w_attn, w_proj, trace=False)
    return full
